# revision 16
# baseline (speedup 1.0000x reference)
"""Chamfer loss kernel for Trainium2, SPMD across 8 NeuronCores.

Problem: target_pc (4, 8192, 3), output_pc (4, 8192, 3) -> scalar chamfer loss
    d2[b,n,m] = |x_n - y_m|^2 ;  dists = sqrt(clip(d2,0)+EPS)
    loss = mean_b( sum_n min_m dists + 2 * sum_m min_n dists )

Sharding: core c handles batch b=c//2 and target-half h=c%2 (M split 2-way).
Each core computes its 8192 x 4096 distance tile:
  - TensorE: K=30 bf16 hi/lo-split feature rows (fp32-grade accuracy at bf16
    speed), N=512 matmuls into two [128,2048] PSUM tiles (4 banks each)
  - ScalarE: one FD=2048 PSUM->SBUF bf16 convert per tile (2x rate; FD=1024
    copies measured 2.4x slower per element)
  - VectorE: col-direction running TT-min (FD=4096 bf16 2x) + row-direction
    min (halving TT tree, or one tensor_tensor_scan), last tree level lands
    in a [128, 64*128] rowbuf; one strided grouped tree finishes all 64
    blocks' row-mins at the end
  - partition-axis mins via TensorE transpose into the same PSUM tiles
  - AllGather of per-core partials; final scalar computed on every core.
"""

import numpy as np


def _ensure_path():
    try:
        import concourse.bass  # noqa: F401
    except ImportError:
        import sys

        for p in ("/opt/trn_rl_repo", "/root/.axon_site/_ro/trn_rl_repo"):
            if p not in sys.path:
                sys.path.insert(0, p)


_ensure_path()

import concourse.bacc as bacc  # noqa: E402
import concourse.tile as tile  # noqa: E402
import concourse.mybir as mybir  # noqa: E402

F32 = mybir.dt.float32
BF16 = mybir.dt.bfloat16
MIN = mybir.AluOpType.min
ADD = mybir.AluOpType.add
SUB = mybir.AluOpType.subtract
MULT = mybir.AluOpType.mult
AX_X = mybir.AxisListType.X
SQRT = mybir.ActivationFunctionType.Sqrt

EPS = 1e-12
N_CORES = 8
K_ROWS = 30
BIGF = 3.0e38


def build_nc(n_pts=8192, m_loc=4096, n_cores=N_CORES, probe_single=False,
             main_repeats=1, whole_repeats=1, row_mode="tree"):
    """Build + compile the SPMD Bass module (one program, runs on all cores)."""
    jx = n_pts // 128
    jy = m_loc // 128
    nblk = n_pts // 128

    nc = bacc.Bacc(
        "TRN2", target_bir_lowering=False, debug=False, num_devices=n_cores
    )

    x_pm_d = nc.dram_tensor("x_pm", [128, jx * 3], F32, kind="ExternalInput")
    y_pm_d = nc.dram_tensor("y_pm", [128, jy * 3], F32, kind="ExternalInput")
    out_d = nc.dram_tensor("out", [1, 1], F32, kind="ExternalOutput")
    eye_d = nc.inline_tensor(np.eye(128, dtype=np.float32), name="eye128")

    with tile.TileContext(nc) as tc:
        with (
            tc.tile_pool(name="const", bufs=1) as constp,
            tc.tile_pool(name="prep", bufs=1) as prep,
            tc.tile_pool(name="feat", bufs=1) as feat,
            tc.tile_pool(name="mm", bufs=2, space="PSUM") as psmm,
            tc.tile_pool(name="stage", bufs=5) as stagep,
            tc.tile_pool(name="tree", bufs=3) as treep,
            tc.tile_pool(name="acc", bufs=1) as accp,
            tc.tile_pool(name="fin", bufs=2) as finp,
            tc.tile_pool(name="dram", bufs=1, space="DRAM") as dramp,
        ):
            x_pm = prep.tile([128, jx * 3], F32, tag="x_pm")
            y_pm = prep.tile([128, jy * 3], F32, tag="y_pm")
            eye_f = constp.tile([128, 128], F32, tag="eye_f")
            nc.sync.dma_start(x_pm[:, :], x_pm_d[:, :])
            nc.sync.dma_start(y_pm[:, :], y_pm_d[:, :])
            nc.sync.dma_start(eye_f[:, :], eye_d.ap())
            eps_c = constp.tile([128, 1], F32, tag="eps_c")
            nc.gpsimd.memset(eps_c[:, :], EPS)
            ones_c = constp.tile([128, 1], F32, tag="ones_c")
            nc.gpsimd.memset(ones_c[:, :], 1.0)

            for _rep in range(whole_repeats):
                _kernel_body(
                    nc, tc, prep, feat, psmm, stagep, treep, accp, finp, dramp,
                    x_pm, y_pm, eye_f, eps_c, ones_c, out_d,
                    n_pts, m_loc, n_cores, jx, jy, nblk,
                    probe_single, main_repeats, row_mode,
                )

    nc.compile()
    return nc


def _features(nc, prep, pm, j, neg2, tagp):
    """hi/lo/lo2 split of (optionally -2x scaled) coords + |p|^2, all bf16."""
    if neg2:
        base = prep.tile([128, j * 3], F32, tag=f"{tagp}_s")
        nc.vector.tensor_scalar_mul(base[:, :], pm[:, :], -2.0)
    else:
        base = pm
    h_b = prep.tile([128, j * 3], BF16, tag=f"{tagp}_hb")
    nc.vector.tensor_copy(h_b[:, :], base[:, :])
    h_f = prep.tile([128, j * 3], F32, tag=f"{tagp}_hf")
    nc.vector.tensor_copy(h_f[:, :], h_b[:, :])
    l_f = prep.tile([128, j * 3], F32, tag=f"{tagp}_lf")
    nc.vector.tensor_tensor(l_f[:, :], base[:, :], h_f[:, :], SUB)
    l_b = prep.tile([128, j * 3], BF16, tag=f"{tagp}_lb")
    nc.vector.tensor_copy(l_b[:, :], l_f[:, :])
    l_f2 = prep.tile([128, j * 3], F32, tag=f"{tagp}_lf2")
    nc.vector.tensor_copy(l_f2[:, :], l_b[:, :])
    lr = prep.tile([128, j * 3], F32, tag=f"{tagp}_lr")
    nc.vector.tensor_tensor(lr[:, :], l_f[:, :], l_f2[:, :], SUB)
    l2_b = prep.tile([128, j * 3], BF16, tag=f"{tagp}_l2b")
    nc.vector.tensor_copy(l2_b[:, :], lr[:, :])
    # squared norm + its split
    sq = prep.tile([128, j * 3], F32, tag=f"{tagp}_sq")
    nc.vector.tensor_tensor(sq[:, :], pm[:, :], pm[:, :], MULT)
    nn = prep.tile([128, j], F32, tag=f"{tagp}_nn")
    nc.vector.tensor_reduce(
        nn[:, :], sq[:, :].rearrange("p (j d) -> p j d", d=3), AX_X, ADD
    )
    nh_b = prep.tile([128, j], BF16, tag=f"{tagp}_nhb")
    nc.vector.tensor_copy(nh_b[:, :], nn[:, :])
    nh_f = prep.tile([128, j], F32, tag=f"{tagp}_nhf")
    nc.vector.tensor_copy(nh_f[:, :], nh_b[:, :])
    nr = prep.tile([128, j], F32, tag=f"{tagp}_nr")
    nc.vector.tensor_tensor(nr[:, :], nn[:, :], nh_f[:, :], SUB)
    nl_b = prep.tile([128, j], BF16, tag=f"{tagp}_nlb")
    nc.vector.tensor_copy(nl_b[:, :], nr[:, :])
    nl_f = prep.tile([128, j], F32, tag=f"{tagp}_nlf")
    nc.vector.tensor_copy(nl_f[:, :], nl_b[:, :])
    nr2 = prep.tile([128, j], F32, tag=f"{tagp}_nr2")
    nc.vector.tensor_tensor(nr2[:, :], nr[:, :], nl_f[:, :], SUB)
    nl2_b = prep.tile([128, j], BF16, tag=f"{tagp}_nl2b")
    nc.vector.tensor_copy(nl2_b[:, :], nr2[:, :])
    return h_b, l_b, l2_b, nh_b, nl_b, nl2_b


def _kernel_body(
    nc, tc, prep, feat, psmm, stagep, treep, accp, finp, dramp,
    x_pm, y_pm, eye_f, eps_c, ones_c, out_d,
    n_pts, m_loc, n_cores, jx, jy, nblk,
    probe_single, main_repeats, row_mode,
):
    xh, xl, xl2, xxh, xxl, xxl2 = _features(nc, prep, x_pm, jx, False, "x")
    yh, yl, yl2, yyh, yyl, yyl2 = _features(nc, prep, y_pm, jy, True, "y")

    # K=30 feature rows; sum_k T[k,m] * X[k,n] == d2[m,n] (bf16 split)
    PX = prep.tile([128, jx * K_ROWS], F32, tag="PX")
    nc.gpsimd.memset(PX[:, :], 1.0)
    PXv = PX[:, :].rearrange("p (j k) -> p j k", k=K_ROWS)
    nc.vector.tensor_copy(PXv[:, :, 3:4], xxh[:, :].rearrange("p (j o) -> p j o", o=1))
    nc.vector.tensor_copy(PXv[:, :, 4:5], xxl[:, :].rearrange("p (j o) -> p j o", o=1))
    nc.vector.tensor_copy(PXv[:, :, 5:6], xxl2[:, :].rearrange("p (j o) -> p j o", o=1))
    xh_v = xh[:, :].rearrange("p (j d) -> p j d", d=3)
    xl_v = xl[:, :].rearrange("p (j d) -> p j d", d=3)
    xl2_v = xl2[:, :].rearrange("p (j d) -> p j d", d=3)
    nc.vector.tensor_copy(PXv[:, :, 6:9], xh_v)
    nc.vector.tensor_copy(PXv[:, :, 9:12], xl_v)
    nc.vector.tensor_copy(PXv[:, :, 12:15], xh_v)
    nc.vector.tensor_copy(PXv[:, :, 15:18], xl_v)
    nc.vector.tensor_copy(PXv[:, :, 18:21], xl2_v)
    nc.vector.tensor_copy(PXv[:, :, 21:24], xh_v)
    nc.vector.tensor_copy(PXv[:, :, 24:27], xl2_v)
    nc.vector.tensor_copy(PXv[:, :, 27:30], xl_v)

    PY = prep.tile([128, jy * K_ROWS], F32, tag="PY")
    nc.gpsimd.memset(PY[:, :], 1.0)
    PYv = PY[:, :].rearrange("p (j k) -> p j k", k=K_ROWS)
    nc.vector.tensor_copy(PYv[:, :, 0:1], yyh[:, :].rearrange("p (j o) -> p j o", o=1))
    nc.vector.tensor_copy(PYv[:, :, 1:2], yyl[:, :].rearrange("p (j o) -> p j o", o=1))
    nc.vector.tensor_copy(PYv[:, :, 2:3], yyl2[:, :].rearrange("p (j o) -> p j o", o=1))
    yh_v = yh[:, :].rearrange("p (j d) -> p j d", d=3)
    yl_v = yl[:, :].rearrange("p (j d) -> p j d", d=3)
    yl2_v = yl2[:, :].rearrange("p (j d) -> p j d", d=3)
    nc.vector.tensor_copy(PYv[:, :, 6:9], yh_v)
    nc.vector.tensor_copy(PYv[:, :, 9:12], yh_v)
    nc.vector.tensor_copy(PYv[:, :, 12:15], yl_v)
    nc.vector.tensor_copy(PYv[:, :, 15:18], yl_v)
    nc.vector.tensor_copy(PYv[:, :, 18:21], yh_v)
    nc.vector.tensor_copy(PYv[:, :, 21:24], yl2_v)
    nc.vector.tensor_copy(PYv[:, :, 24:27], yl_v)
    nc.vector.tensor_copy(PYv[:, :, 27:30], yl2_v)

    # ------- transpose to feature-major [K, points], batched via PSUM -------
    X_sb = feat.tile([K_ROWS, n_pts], BF16, tag="X_sb")
    T_sb = feat.tile([K_ROWS, m_loc], BF16, tag="T_sb")

    def tp_group(P, dst, g):
        mmt = psmm.tile([128, 2048], F32, tag="mm")
        for i in range(16):
            j = 16 * g + i
            nc.tensor.transpose(
                mmt[0:K_ROWS, 128 * i : 128 * (i + 1)],
                P[:, K_ROWS * j : K_ROWS * (j + 1)],
                eye_f[:, :],
            )
        nc.scalar.copy(dst[:, 2048 * g : 2048 * (g + 1)], mmt[0:K_ROWS, 0:2048])

    for g in range(jy // 16):
        tp_group(PY, T_sb, g)
    tp_group(PX, X_sb, 0)

    # ---------------- main distance + min loop ----------------
    colrun = accp.tile([128, m_loc], BF16, tag="colrun")
    rowbuf = accp.tile([128, nblk * 128], BF16, tag="rowbuf")
    rowminsq = accp.tile([128, nblk], BF16, tag="rowminsq")

    def grouped_tree(lo, hi):
        # [128, (hi-lo blocks, 128)] -> rowminsq[:, lo:hi]
        w = 128
        rv = rowbuf[:, 128 * lo : 128 * hi].rearrange("p (b w) -> p b w", w=128)
        while w > 1:
            h = w // 2
            nc.vector.tensor_tensor(rv[:, :, 0:h], rv[:, :, 0:h], rv[:, :, h:w], MIN)
            w = h
        rmv = rowminsq[:, lo:hi].rearrange("p (b o) -> p b o", o=1)
        nc.vector.tensor_copy(rmv[:, :, :], rv[:, :, 0:1])

    for nb in [i for _ in range(main_repeats) for i in range(nblk)]:
        if nb % 16 == 0 and nb > 0 and nb // 16 < jx // 16:
            tp_group(PX, X_sb, nb // 16)
        if nb == nblk // 2:
            grouped_tree(0, nblk // 2)
        lhs = X_sb[:, 128 * nb : 128 * (nb + 1)]
        stage = stagep.tile([128, m_loc], BF16, tag="stage")
        for half in range(2):
            pmm = psmm.tile([128, 2048], F32, tag="mm")
            for q in range(4):
                nc.tensor.matmul(
                    pmm[:, 512 * q : 512 * (q + 1)],
                    lhs,
                    T_sb[:, 2048 * half + 512 * q : 2048 * half + 512 * (q + 1)],
                    start=True,
                    stop=True,
                )
            nc.scalar.copy(
                stage[:, 2048 * half : 2048 * (half + 1)], pmm[:, :]
            )
        # col direction: accumulate over nb (elementwise in m)
        if nb == 0:
            nc.vector.tensor_copy(colrun[:, :], stage[:, :])
        else:
            nc.vector.tensor_tensor(colrun[:, :], colrun[:, :], stage[:, :], MIN)
        # row direction: T1 into per-pair slot; batched lower levels
        half_slot = nb % 2
        if half_slot == 0:
            tr_pair = treep.tile([128, m_loc], BF16, tag="tree")
        tr = tr_pair
        nc.vector.tensor_tensor(
            tr[:, 2048 * half_slot : 2048 * (half_slot + 1)],
            stage[:, 0 : m_loc // 2],
            stage[:, m_loc // 2 : m_loc],
            MIN,
        )
        if row_mode == "gp2":
            # per-block contiguous lower levels on GPSIMD
            t0 = 2048 * half_slot
            w = 2048
            while w > 256:
                h = w // 2
                nc.gpsimd.tensor_tensor(
                    tr[:, t0 : t0 + h], tr[:, t0 : t0 + h], tr[:, t0 + h : t0 + w], MIN
                )
                w = h
            nc.gpsimd.tensor_tensor(
                rowbuf[:, 128 * nb : 128 * (nb + 1)],
                tr[:, t0 : t0 + 128],
                tr[:, t0 + 128 : t0 + 256],
                MIN,
            )
            continue
        if half_slot == 1:
            # both blocks' T1 in tr: batched halving via 3D AP, one op/level
            tv = tr[:, :].rearrange("p (g w) -> p g w", w=2048)
            eng = nc.vector if row_mode != "gp" else nc.gpsimd
            nc.vector.tensor_tensor(
                tv[:, :, 0:1024], tv[:, :, 0:1024], tv[:, :, 1024:2048], MIN
            )
            w = 1024
            while w > 256:
                h = w // 2
                eng.tensor_tensor(tv[:, :, 0:h], tv[:, :, 0:h], tv[:, :, h:w], MIN)
                w = h
            rb2 = rowbuf[:, 128 * (nb - 1) : 128 * (nb + 1)].rearrange(
                "p (g w) -> p g w", w=128
            )
            eng.tensor_tensor(rb2[:, :, :], tv[:, :, 0:128], tv[:, :, 128:256], MIN)

    grouped_tree(nblk // 2, nblk)

    # ---------------- col-direction finale ----------------
    colrun_f = accp.tile([128, m_loc], F32, tag="colrun_f")
    for h in range(2):
        nc.scalar.copy(
            colrun_f[:, 2048 * h : 2048 * (h + 1)],
            colrun[:, 2048 * h : 2048 * (h + 1)],
        )
    colminsq = finp.tile([128, jy], F32, tag="colminsq")
    for g in range(jy // 16):
        mmt = psmm.tile([128, 2048], F32, tag="mm")
        for i in range(16):
            c = 16 * g + i
            nc.tensor.transpose(
                mmt[:, 128 * i : 128 * (i + 1)],
                colrun_f[:, 128 * c : 128 * (c + 1)],
                eye_f[:, :],
            )
        nc.vector.tensor_reduce(
            colminsq[:, 16 * g : 16 * (g + 1)],
            mmt[:, :].rearrange("p (q c) -> p q c", c=128),
            AX_X,
            MIN,
        )

    # local col finish: clip, sqrt(+EPS), sum over local m
    nc.vector.tensor_scalar_max(colminsq[:, :], colminsq[:, :], 0.0)
    colsq = finp.tile([128, jy], F32, tag="colsq")
    nc.scalar.activation(colsq[:, :], colminsq[:, :], SQRT, bias=eps_c[:, :])
    colsum = finp.tile([128, 1], F32, tag="colsum")
    nc.vector.tensor_reduce(colsum[:, :], colsq[:, :], AX_X, ADD)

    # ---------------- collective: AllGather partials ----------------
    W = nblk + 1
    pay = finp.tile([128, W], F32, tag="pay")
    nc.vector.tensor_copy(pay[:, 0:nblk], rowminsq[:, :])
    nc.vector.tensor_copy(pay[:, nblk:W], colsum[:, :])
    if probe_single:
        nc.sync.dma_start(out_d[:, :], pay[0:1, 0:1])
        return
    cc_in = dramp.tile([128, W], F32, tag="cc_in")
    cc_out = dramp.tile([128 * n_cores, W], F32, tag="cc_out")
    nc.sync.dma_start(cc_in[:, :], pay[:, :])
    nc.gpsimd.collective_compute(
        "AllGather",
        mybir.AluOpType.bypass,
        replica_groups=[list(range(n_cores))],
        ins=[cc_in.opt()],
        outs=[cc_out.opt()],
    )

    # ---------------- final scalar (same on every core) -------------
    ga = finp.tile([128, n_cores * W], F32, tag="ga")
    for c in range(n_cores):
        nc.sync.dma_start(
            ga[:, W * c : W * (c + 1)], cc_out[128 * c : 128 * (c + 1), :]
        )
    gav = ga[:, :].rearrange("p (c w) -> p c w", w=W)
    # rows: partner-min over core pairs (same batch, two target halves)
    pm = finp.tile([128, (n_cores // 2) * nblk], F32, tag="pm")
    pmv = pm[:, :].rearrange("p (b w) -> p b w", w=nblk)
    for b in range(n_cores // 2):
        nc.vector.tensor_tensor(
            pmv[:, b : b + 1, :],
            gav[:, 2 * b : 2 * b + 1, 0:nblk],
            gav[:, 2 * b + 1 : 2 * b + 2, 0:nblk],
            MIN,
        )
    nc.vector.tensor_scalar_max(pm[:, :], pm[:, :], 0.0)
    sq = finp.tile([128, (n_cores // 2) * nblk], F32, tag="sqf")
    nc.scalar.activation(sq[:, :], pm[:, :], SQRT, bias=eps_c[:, :])
    a2b = finp.tile([128, 1], F32, tag="a2b")
    nc.vector.tensor_reduce(a2b[:, :], sq[:, :], AX_X, ADD)
    b2a8 = finp.tile([128, n_cores], F32, tag="b2a8")
    nc.vector.tensor_reduce(b2a8[:, :], gav[:, :, nblk : nblk + 1], AX_X, ADD)
    b2a = finp.tile([128, 1], F32, tag="b2a")
    nc.vector.tensor_reduce(b2a[:, :], b2a8[:, :], AX_X, ADD)

    # loss = (1/B) * (sum_b a2b + 2 * sum_b b2a); B = n_cores/2
    inv_b = 2.0 / n_cores
    t1 = finp.tile([128, 1], F32, tag="t1")
    nc.vector.tensor_scalar_mul(t1[:, :], a2b[:, :], inv_b)
    t2 = finp.tile([128, 1], F32, tag="t2")
    nc.vector.tensor_scalar_mul(t2[:, :], b2a[:, :], 2.0 * inv_b)
    t3 = finp.tile([128, 1], F32, tag="t3")
    nc.vector.tensor_tensor(t3[:, :], t1[:, :], t2[:, :], ADD)
    ps_l = psmm.tile([128, 2048], F32, tag="mm")
    nc.tensor.matmul(ps_l[0:1, 0:1], t3[:, :], ones_c[:, :], start=True, stop=True)
    loss = finp.tile([1, 1], F32, tag="loss")
    nc.scalar.copy(loss[:, :], ps_l[0:1, 0:1])
    nc.sync.dma_start(out_d[:, :], loss[:, :])


def shard_inputs(target_pc, output_pc, n_cores=N_CORES):
    """Pure-layout host-side sharding: core c gets batch c//2, target half c%2."""
    tp = np.ascontiguousarray(np.asarray(target_pc, dtype=np.float32))
    op = np.ascontiguousarray(np.asarray(output_pc, dtype=np.float32))
    B, M, D = tp.shape
    _, N, _ = op.shape
    assert D == 3 and B == n_cores // 2
    m_loc = M // 2
    in_maps = []
    for c in range(n_cores):
        b, h = c // 2, c % 2
        x = op[b]
        y = tp[b, h * m_loc : (h + 1) * m_loc]
        in_maps.append(
            {
                "x_pm": np.ascontiguousarray(x.reshape(128, -1)),
                "y_pm": np.ascontiguousarray(y.reshape(128, -1)),
            }
        )
    return in_maps, N, m_loc


_NC_CACHE = {}


def _get_nc(n_pts, m_loc):
    key = (n_pts, m_loc)
    if key not in _NC_CACHE:
        _NC_CACHE[key] = build_nc(n_pts=n_pts, m_loc=m_loc)
    return _NC_CACHE[key]


def kernel(target_pc=None, output_pc=None, **_unused):
    from concourse.bass_utils import run_bass_kernel_spmd

    in_maps, n_pts, m_loc = shard_inputs(target_pc, output_pc)
    nc = _get_nc(n_pts, m_loc)
    res = run_bass_kernel_spmd(nc, in_maps, core_ids=list(range(N_CORES)))
    out = np.asarray(res.results[0]["out"], dtype=np.float32)
    return np.float32(out.reshape(()))
